# revision 2
# baseline (speedup 1.0000x reference)
"""Trainium2 Bass kernel for nn_NodeNetwork (GNN message passing).

Computation (per batch b):
    bo = Ro^T X            [E, D]   gather  (contract n)
    bi = Ri^T X            [E, D]   gather  (contract n)
    mi = (Ri . e) bo       [N, D]   scatter (contract e)
    mo = (Ro . e) bi       [N, D]   scatter (contract e)
    h  = tanh([mi, mo, X] @ W1 + b1)
    y  = tanh(h @ W2 + b2)

Sharding: 8 cores = 2 batches x 4 edge-shards (ESH = E/4 = 4096 per core).
Each core computes bo/bi for its edge shard (full N), then the partial
mi/mo contribution of its edges, folds the (linear) first MLP layer into a
z-partial, ReduceScatters z over the 4-core batch group (each core ends up
with one N/4 slice), and finishes the MLP on its slice.

Layouts: the incidence slices are shipped in fp16 in BOTH orientations
(straight [N, ESH] for the n-contraction, transposed [ESH, N] for the
e-contraction) - same total bytes as one fp32 copy.  One-hot 0/1 incidence
entries are exact in fp16; X is fp16 for the gather (rel err ~5e-4), all
accumulation is fp32 in PSUM and the MLP runs in fp32.
"""

import numpy as np

import concourse.bass as bass
import concourse.mybir as mybir
import concourse.tile as tile
from concourse import bacc
from concourse.bass_utils import run_bass_kernel_spmd
from concourse.masks import make_identity

B, N, E, D, OUT = 2, 4096, 16384, 64, 64
NCORES = 8
G = 4              # cores per batch
ESH = E // G       # edges per core
NB = N // 128      # 32 n-blocks
EQ = ESH // 1024   # 4 e-quads (DMA granularity)
NQ = N // 1024     # 4 n-quads
NSL = N // G       # 1024 output n-slice per core

F32 = mybir.dt.float32
F16 = mybir.dt.float16

_cache = {}


def _build_program():
    nc = bacc.Bacc(
        "TRN2",
        target_bir_lowering=False,
        debug=False,
        num_devices=NCORES,
    )

    ris = nc.declare_dram_parameter("ris", [N, ESH], F16, isOutput=False)
    ros = nc.declare_dram_parameter("ros", [N, ESH], F16, isOutput=False)
    rit = nc.declare_dram_parameter("rit", [ESH, N], F16, isOutput=False)
    rot = nc.declare_dram_parameter("rot", [ESH, N], F16, isOutput=False)
    x16r = nc.declare_dram_parameter("x16r", [128, NB * D], F16, isOutput=False)
    xt2 = nc.declare_dram_parameter("xt2", [128, N], F32, isOutput=False)
    earr = nc.declare_dram_parameter("earr", [128, ESH // 128], F32, isOutput=False)
    w1ab = nc.declare_dram_parameter("w1ab", [128, OUT], F32, isOutput=False)
    w1cp = nc.declare_dram_parameter("w1cp", [128, OUT], F32, isOutput=False)
    w2p = nc.declare_dram_parameter("w2p", [128, OUT], F32, isOutput=False)
    b1d = nc.declare_dram_parameter("b1d", [OUT, 1], F32, isOutput=False)
    b2d = nc.declare_dram_parameter("b2d", [OUT, 1], F32, isOutput=False)
    out = nc.declare_dram_parameter("out", [OUT, NSL], F32, isOutput=True)

    with tile.TileContext(nc) as tc:
        with (
            tc.tile_pool(name="const", bufs=1) as cpool,
            tc.tile_pool(name="stream", bufs=4) as spool,
            tc.tile_pool(name="stage", bufs=4) as stpool,
            tc.tile_pool(name="dram", bufs=1, space="DRAM") as dpool,
        ):
            # ---- constants / small inputs ----
            x16_sb = cpool.tile([128, NB, D], F16)
            nc.sync.dma_start(x16_sb[:], x16r.rearrange("p (nb d) -> p nb d", d=D))
            xt2_sb = cpool.tile([128, N], F32)
            nc.sync.dma_start(xt2_sb[:], xt2[:])
            e_sb = cpool.tile([128, ESH // 128], F32)
            nc.sync.dma_start(e_sb[:], earr[:])
            w1ab_sb = cpool.tile([128, OUT], F32)
            nc.sync.dma_start(w1ab_sb[:], w1ab[:])
            w1cp_sb = cpool.tile([128, OUT], F32)
            nc.sync.dma_start(w1cp_sb[:], w1cp[:])
            w2p_sb = cpool.tile([128, OUT], F32)
            nc.sync.dma_start(w2p_sb[:], w2p[:])
            b1_sb = cpool.tile([OUT, 1], F32)
            nc.sync.dma_start(b1_sb[:], b1d[:])
            b2_sb = cpool.tile([OUT, 1], F32)
            nc.sync.dma_start(b2_sb[:], b2d[:])
            id16 = cpool.tile([64, 64], F16)
            make_identity(nc, id16[:])

            # persistent per-edge-block gathered features (bo', bi'), fp16
            bo_blk = cpool.tile([128, ESH // 128, D], F16)
            bi_blk = cpool.tile([128, ESH // 128, D], F16)

            # collective bounce buffers
            zpart = dpool.tile([G * OUT, NSL], F32)
            zred = dpool.tile([OUT, NSL], F32)

            # ---- phase B: gathers (contract n) ----
            with (
                tc.tile_pool(name="psg", bufs=4, space="PSUM") as pg,
                tc.tile_pool(name="pst", bufs=2, space="PSUM") as pt,
            ):
                for q in range(EQ):
                    ps_bo = [pg.tile([64, 512], F32, tag="psg", name=f"ps_bo{q}_{i}") for i in range(2)]
                    ps_bi = [pg.tile([64, 512], F32, tag="psg", name=f"ps_bi{q}_{i}") for i in range(2)]
                    for nb in range(NB):
                        ro_t = spool.tile([128, 1024], F16, tag="ro")
                        nc.sync.dma_start(
                            ro_t[:],
                            ros[nb * 128 : (nb + 1) * 128, q * 1024 : (q + 1) * 1024],
                        )
                        ri_t = spool.tile([128, 1024], F16, tag="ri")
                        nc.sync.dma_start(
                            ri_t[:],
                            ris[nb * 128 : (nb + 1) * 128, q * 1024 : (q + 1) * 1024],
                        )
                        st, sp = (nb == 0), (nb == NB - 1)
                        for h in range(2):
                            sl = slice(h * 512, (h + 1) * 512)
                            nc.tensor.matmul(
                                ps_bo[h], x16_sb[:, nb, :], ro_t[:, sl],
                                start=st, stop=sp,
                            )
                            nc.tensor.matmul(
                                ps_bi[h], x16_sb[:, nb, :], ri_t[:, sl],
                                start=st, stop=sp,
                            )
                    # epilogue: cast to fp16, transpose to [e,128 x d], e-scale
                    for h in range(2):
                        for name, ps, blk in (
                            ("bo", ps_bo[h], bo_blk),
                            ("bi", ps_bi[h], bi_blk),
                        ):
                            bt16 = stpool.tile([64, 512], F16, tag="bt16")
                            nc.vector.tensor_copy(bt16[:], ps)
                            for t in range(4):
                                eb = (2 * q + h) * 4 + t
                                ps_t = pt.tile([128, 64], F16, tag="pst")
                                nc.tensor.transpose(
                                    ps_t[:], bt16[:, t * 128 : (t + 1) * 128], id16[:]
                                )
                                nc.vector.tensor_scalar_mul(
                                    blk[:, eb, :], ps_t[:], e_sb[:, eb : eb + 1]
                                )

            # ---- phase C: scatters (contract e) + z-fold ----
            with (
                tc.tile_pool(name="pss", bufs=4, space="PSUM") as ps_pool,
                tc.tile_pool(name="psz", bufs=2, space="PSUM") as pz_pool,
            ):
                for q in range(NQ):
                    ps_mi = [ps_pool.tile([64, 512], F32, tag="pss", name=f"ps_mi{q}_{i}") for i in range(2)]
                    ps_mo = [ps_pool.tile([64, 512], F32, tag="pss", name=f"ps_mo{q}_{i}") for i in range(2)]
                    for eb in range(ESH // 128):
                        rit_t = spool.tile([128, 1024], F16, tag="rit")
                        nc.sync.dma_start(
                            rit_t[:],
                            rit[eb * 128 : (eb + 1) * 128, q * 1024 : (q + 1) * 1024],
                        )
                        rot_t = spool.tile([128, 1024], F16, tag="rot")
                        nc.sync.dma_start(
                            rot_t[:],
                            rot[eb * 128 : (eb + 1) * 128, q * 1024 : (q + 1) * 1024],
                        )
                        st, sp = (eb == 0), (eb == ESH // 128 - 1)
                        for h in range(2):
                            sl = slice(h * 512, (h + 1) * 512)
                            nc.tensor.matmul(
                                ps_mi[h], bo_blk[:, eb, :], rit_t[:, sl],
                                start=st, stop=sp,
                            )
                            nc.tensor.matmul(
                                ps_mo[h], bi_blk[:, eb, :], rot_t[:, sl],
                                start=st, stop=sp,
                            )
                    for h in range(2):
                        mm = stpool.tile([128, 512], F32, tag="mimo")
                        nc.vector.tensor_copy(mm[:64, :], ps_mi[h])
                        nc.vector.tensor_copy(mm[64:, :], ps_mo[h])
                        pz = pz_pool.tile([64, 512], F32, tag="psz")
                        nc.tensor.matmul(pz, w1ab_sb[:], mm[:], start=True, stop=False)
                        nc.tensor.matmul(
                            pz, w1cp_sb[:],
                            xt2_sb[:, q * 1024 + h * 512 : q * 1024 + (h + 1) * 512],
                            start=False, stop=True,
                        )
                        zsb = stpool.tile([64, 512], F32, tag="zsb")
                        nc.vector.tensor_copy(zsb[:], pz)
                        nc.sync.dma_start(
                            zpart[q * OUT : (q + 1) * OUT, h * 512 : (h + 1) * 512],
                            zsb[:],
                        )

            # ---- phase D: cross-core reduction over the 4-core batch group ----
            nc.gpsimd.collective_compute(
                "ReduceScatter",
                mybir.AluOpType.add,
                replica_groups=[[0, 1, 2, 3], [4, 5, 6, 7]],
                ins=[zpart.opt()],
                outs=[zred.opt()],
            )

            # ---- phase E: finish MLP on this core's n-slice ----
            with tc.tile_pool(name="psy", bufs=2, space="PSUM") as py_pool:
                zred_sb = stpool.tile([OUT, NSL], F32, tag="zred")
                nc.sync.dma_start(zred_sb[:], zred[:])
                h2 = stpool.tile([128, NSL], F32, tag="h2")
                nc.scalar.activation(
                    h2[:64, :], zred_sb[:], mybir.ActivationFunctionType.Tanh,
                    bias=b1_sb[:],
                )
                nc.scalar.activation(
                    h2[64:, :], zred_sb[:], mybir.ActivationFunctionType.Tanh,
                    bias=b1_sb[:],
                )
                for h in range(2):
                    sl = slice(h * 512, (h + 1) * 512)
                    py = py_pool.tile([64, 512], F32, tag="psy")
                    nc.tensor.matmul(py, w2p_sb[:], h2[:, sl], start=True, stop=True)
                    ysb = stpool.tile([64, 512], F32, tag="ysb")
                    nc.scalar.activation(
                        ysb[:], py, mybir.ActivationFunctionType.Tanh, bias=b2_sb[:]
                    )
                    nc.sync.dma_start(out[:, sl], ysb[:])

    nc.compile()
    return nc


def make_in_maps(X, e, Ri, Ro, W1, b1, W2, b2):
    """Shard + lay out the full inputs for the 8 cores."""
    X = np.asarray(X, dtype=np.float32)
    e = np.asarray(e, dtype=np.float32)
    W1 = np.asarray(W1, dtype=np.float32)
    b1 = np.asarray(b1, dtype=np.float32)
    W2 = np.asarray(W2, dtype=np.float32)
    b2 = np.asarray(b2, dtype=np.float32)

    w1ab = np.ascontiguousarray(W1[:128])                       # [128, OUT]
    w1cp = np.concatenate([W1[128:], np.zeros((64, OUT), np.float32)], axis=0)
    w2p = np.concatenate([W2, np.zeros((64, OUT), np.float32)], axis=0)
    b1c = np.ascontiguousarray(b1.reshape(OUT, 1))
    b2c = np.ascontiguousarray(b2.reshape(OUT, 1))

    in_maps = []
    per_batch = {}
    for b_ in range(B):
        xb = np.asarray(X[b_])
        x16 = xb.astype(np.float16)
        # [p, nb, d] layout so the DMA is fully contiguous
        x16r = np.ascontiguousarray(
            x16.reshape(NB, 128, D).transpose(1, 0, 2)
        ).reshape(128, NB * D)
        xt = np.ascontiguousarray(xb.T)                         # [D, N] f32
        xt2 = np.concatenate([xt, xt], axis=0) * 0.25           # [128, N]
        per_batch[b_] = (x16r, xt2)

    for c in range(NCORES):
        b_, s = divmod(c, G)
        sl = slice(s * ESH, (s + 1) * ESH)
        ri_sl = np.asarray(Ri[b_, :, sl], dtype=np.float32)
        ro_sl = np.asarray(Ro[b_, :, sl], dtype=np.float32)
        ris = ri_sl.astype(np.float16)
        ros = ro_sl.astype(np.float16)
        ritm = np.ascontiguousarray(ris.T)
        rotm = np.ascontiguousarray(ros.T)
        e_sl = np.asarray(e[b_, sl], dtype=np.float32)
        earr = np.ascontiguousarray(e_sl.reshape(ESH // 128, 128).T)
        x16r, xt2 = per_batch[b_]
        in_maps.append(
            {
                "ris": ris, "ros": ros, "rit": ritm, "rot": rotm,
                "x16r": x16r, "xt2": xt2, "earr": earr,
                "w1ab": w1ab, "w1cp": w1cp, "w2p": w2p,
                "b1d": b1c, "b2d": b2c,
            }
        )
    return in_maps


def assemble_output(results):
    y = np.empty((B, N, OUT), dtype=np.float32)
    for c in range(NCORES):
        b_, s = divmod(c, G)
        y[b_, s * NSL : (s + 1) * NSL, :] = results[c]["out"].T
    return y


def get_program():
    if "nc" not in _cache:
        _cache["nc"] = _build_program()
    return _cache["nc"]


def kernel(X, e, Ri, Ro, W1, b1, W2, b2):
    nc = get_program()
    in_maps = make_in_maps(X, e, Ri, Ro, W1, b1, W2, b2)
    res = run_bass_kernel_spmd(nc, in_maps, list(range(NCORES)))
    return assemble_output(res.results)


# revision 4
# speedup vs baseline: 147.7996x; 147.7996x over previous
"""Trainium2 Bass kernel for nn_NodeNetwork (GNN message passing).

Computation (per batch b):
    bo = Ro^T X            [E, D]   gather  (contract n)
    bi = Ri^T X            [E, D]   gather  (contract n)
    mi = (Ri . e) bo       [N, D]   scatter (contract e)
    mo = (Ro . e) bi       [N, D]   scatter (contract e)
    h  = tanh([mi, mo, X] @ W1 + b1)
    y  = tanh(h @ W2 + b2)

Sharding: 8 cores = 2 batches x 4 edge-shards (ESH = E/4 = 4096 per core).
Each core computes bo/bi for its edge shard (full N), then the partial
mi/mo contribution of its edges, folds the (linear) first MLP layer into a
z-partial, ReduceScatters z over the 4-core batch group (each core ends up
with one N/4 slice), and finishes the MLP on its slice.

Layouts: the incidence slices are shipped in fp16 in BOTH orientations
(straight [N, ESH] for the n-contraction, transposed [ESH, N] for the
e-contraction) - same total bytes as one fp32 copy.  One-hot 0/1 incidence
entries are exact in fp16; X is fp16 for the gather (rel err ~5e-4), all
accumulation is fp32 in PSUM and the MLP runs in fp32.
"""

import numpy as np

import concourse.bass as bass
import concourse.mybir as mybir
import concourse.tile as tile
from concourse import bacc
from concourse.bass_utils import run_bass_kernel_spmd
from concourse.masks import make_identity

B, N, E, D, OUT = 2, 4096, 16384, 64, 64
NCORES = 8
G = 4              # cores per batch
ESH = E // G       # edges per core
NB = N // 128      # 32 n-blocks
EQ = ESH // 1024   # 4 e-quads (DMA granularity)
NQ = N // 1024     # 4 n-quads
NSL = N // G       # 1024 output n-slice per core

F32 = mybir.dt.float32
F16 = mybir.dt.float16

_cache = {}


def _build_program(repeat=1):
    nc = bacc.Bacc(
        "TRN2",
        target_bir_lowering=False,
        debug=False,
        num_devices=NCORES,
    )

    ris = nc.declare_dram_parameter("ris", [N, ESH], F16, isOutput=False)
    ros = nc.declare_dram_parameter("ros", [N, ESH], F16, isOutput=False)
    rit = nc.declare_dram_parameter("rit", [ESH, N], F16, isOutput=False)
    rot = nc.declare_dram_parameter("rot", [ESH, N], F16, isOutput=False)
    x16r = nc.declare_dram_parameter("x16r", [128, NB * D], F16, isOutput=False)
    xt2 = nc.declare_dram_parameter("xt2", [128, N], F32, isOutput=False)
    earr = nc.declare_dram_parameter("earr", [128, ESH // 128], F32, isOutput=False)
    w1ab = nc.declare_dram_parameter("w1ab", [128, OUT], F32, isOutput=False)
    w1cp = nc.declare_dram_parameter("w1cp", [128, OUT], F32, isOutput=False)
    w2p = nc.declare_dram_parameter("w2p", [128, OUT], F32, isOutput=False)
    b1d = nc.declare_dram_parameter("b1d", [OUT, 1], F32, isOutput=False)
    b2d = nc.declare_dram_parameter("b2d", [OUT, 1], F32, isOutput=False)
    out = nc.declare_dram_parameter("out", [OUT, NSL], F32, isOutput=True)

    with tile.TileContext(nc) as tc:
        with (
            tc.tile_pool(name="const", bufs=1) as cpool,
            tc.tile_pool(name="stream", bufs=4) as spool,
            tc.tile_pool(name="stage", bufs=4) as stpool,
            tc.tile_pool(name="psum", bufs=7, space="PSUM") as ppool,
            tc.tile_pool(name="dram", bufs=1, space="DRAM") as dpool,
        ):
            # ---- constants / small inputs ----
            x16_sb = cpool.tile([128, NB, D], F16)
            nc.sync.dma_start(x16_sb[:], x16r.rearrange("p (nb d) -> p nb d", d=D))
            xt2_sb = cpool.tile([128, N], F32)
            nc.sync.dma_start(xt2_sb[:], xt2[:])
            e_sb = cpool.tile([128, ESH // 128], F32)
            nc.sync.dma_start(e_sb[:], earr[:])
            w1ab_sb = cpool.tile([128, OUT], F32)
            nc.sync.dma_start(w1ab_sb[:], w1ab[:])
            w1cp_sb = cpool.tile([128, OUT], F32)
            nc.sync.dma_start(w1cp_sb[:], w1cp[:])
            w2p_sb = cpool.tile([128, OUT], F32)
            nc.sync.dma_start(w2p_sb[:], w2p[:])
            b1_sb = cpool.tile([OUT, 1], F32)
            nc.sync.dma_start(b1_sb[:], b1d[:])
            b2_sb = cpool.tile([OUT, 1], F32)
            nc.sync.dma_start(b2_sb[:], b2d[:])
            id16 = cpool.tile([64, 64], F16)
            make_identity(nc, id16[:])

            # persistent per-edge-block gathered features (bo', bi'), fp16
            bo_blk = cpool.tile([128, ESH // 128, D], F16)
            bi_blk = cpool.tile([128, ESH // 128, D], F16)

            # collective bounce buffers
            zpart = dpool.tile([G * OUT, NSL], F32)
            zred = dpool.tile([OUT, NSL], F32)

            def body_bc(_i=None):
                # ---- phase B: gathers (contract n) ----
                for q in range(EQ):
                    ps_bo = [ppool.tile([64, 512], F32, tag="ps", name=f"ps_bo{q}_{i}") for i in range(2)]
                    ps_bi = [ppool.tile([64, 512], F32, tag="ps", name=f"ps_bi{q}_{i}") for i in range(2)]
                    for nb in range(NB):
                        ro_t = spool.tile([128, 1024], F16, tag="ro", name="ro_t")
                        nc.sync.dma_start(
                            ro_t[:],
                            ros[nb * 128 : (nb + 1) * 128, q * 1024 : (q + 1) * 1024],
                        )
                        ri_t = spool.tile([128, 1024], F16, tag="ri", name="ri_t")
                        nc.sync.dma_start(
                            ri_t[:],
                            ris[nb * 128 : (nb + 1) * 128, q * 1024 : (q + 1) * 1024],
                        )
                        st, sp = (nb == 0), (nb == NB - 1)
                        for h in range(2):
                            sl = slice(h * 512, (h + 1) * 512)
                            nc.tensor.matmul(
                                ps_bo[h], x16_sb[:, nb, :], ro_t[:, sl],
                                start=st, stop=sp,
                            )
                            nc.tensor.matmul(
                                ps_bi[h], x16_sb[:, nb, :], ri_t[:, sl],
                                start=st, stop=sp,
                            )
                    # epilogue: cast to fp16, transpose to [e128 x d], e-scale
                    for h in range(2):
                        for ps, blk in ((ps_bo[h], bo_blk), (ps_bi[h], bi_blk)):
                            bt16 = stpool.tile([64, 512], F16, tag="bt16", name="bt16")
                            nc.vector.tensor_copy(bt16[:], ps)
                            for t in range(4):
                                eb = (2 * q + h) * 4 + t
                                ps_t = ppool.tile([128, 64], F16, tag="ps", name="ps_t")
                                nc.tensor.transpose(
                                    ps_t[:, :64], bt16[:, t * 128 : (t + 1) * 128], id16[:]
                                )
                                nc.vector.tensor_scalar_mul(
                                    blk[:, eb, :], ps_t[:, :64], e_sb[:, eb : eb + 1]
                                )

                # ---- phase C: scatters (contract e) + z-fold ----
                for q in range(NQ):
                    ps_mi = [ppool.tile([64, 512], F32, tag="ps", name=f"ps_mi{q}_{i}") for i in range(2)]
                    ps_mo = [ppool.tile([64, 512], F32, tag="ps", name=f"ps_mo{q}_{i}") for i in range(2)]
                    for eb in range(ESH // 128):
                        rit_t = spool.tile([128, 1024], F16, tag="rit", name="rit_t")
                        nc.sync.dma_start(
                            rit_t[:],
                            rit[eb * 128 : (eb + 1) * 128, q * 1024 : (q + 1) * 1024],
                        )
                        rot_t = spool.tile([128, 1024], F16, tag="rot", name="rot_t")
                        nc.sync.dma_start(
                            rot_t[:],
                            rot[eb * 128 : (eb + 1) * 128, q * 1024 : (q + 1) * 1024],
                        )
                        st, sp = (eb == 0), (eb == ESH // 128 - 1)
                        for h in range(2):
                            sl = slice(h * 512, (h + 1) * 512)
                            nc.tensor.matmul(
                                ps_mi[h], bo_blk[:, eb, :], rit_t[:, sl],
                                start=st, stop=sp,
                            )
                            nc.tensor.matmul(
                                ps_mo[h], bi_blk[:, eb, :], rot_t[:, sl],
                                start=st, stop=sp,
                            )
                    for h in range(2):
                        mm = stpool.tile([128, 512], F32, tag="mimo", name="mm")
                        nc.vector.tensor_copy(mm[:64, :], ps_mi[h])
                        nc.vector.tensor_copy(mm[64:, :], ps_mo[h])
                        pz = ppool.tile([64, 512], F32, tag="ps", name="pz")
                        nc.tensor.matmul(pz, w1ab_sb[:], mm[:], start=True, stop=False)
                        nc.tensor.matmul(
                            pz, w1cp_sb[:],
                            xt2_sb[:, q * 1024 + h * 512 : q * 1024 + (h + 1) * 512],
                            start=False, stop=True,
                        )
                        zsb = stpool.tile([64, 512], F32, tag="zsb", name="zsb")
                        nc.vector.tensor_copy(zsb[:], pz)
                        nc.sync.dma_start(
                            zpart[q * OUT : (q + 1) * OUT, h * 512 : (h + 1) * 512],
                            zsb[:],
                        )

            def tail():
                # ---- phase D: cross-core reduction over the 4-core batch group ----
                nc.gpsimd.collective_compute(
                    "ReduceScatter",
                    mybir.AluOpType.add,
                    replica_groups=[[0, 1, 2, 3], [4, 5, 6, 7]],
                    ins=[zpart.opt()],
                    outs=[zred.opt()],
                )

                # ---- phase E: finish MLP on this core's n-slice ----
                zred_sb = stpool.tile([OUT, NSL], F32, tag="zred", name="zred_sb")
                nc.sync.dma_start(zred_sb[:], zred[:])
                h2 = stpool.tile([128, NSL], F32, tag="h2", name="h2")
                nc.scalar.activation(
                    h2[:64, :], zred_sb[:], mybir.ActivationFunctionType.Tanh,
                    bias=b1_sb[:],
                )
                nc.scalar.activation(
                    h2[64:, :], zred_sb[:], mybir.ActivationFunctionType.Tanh,
                    bias=b1_sb[:],
                )
                for h in range(2):
                    sl = slice(h * 512, (h + 1) * 512)
                    py = ppool.tile([64, 512], F32, tag="ps", name="py")
                    nc.tensor.matmul(py, w2p_sb[:], h2[:, sl], start=True, stop=True)
                    ysb = stpool.tile([64, 512], F32, tag="ysb", name="ysb")
                    nc.scalar.activation(
                        ysb[:], py, mybir.ActivationFunctionType.Tanh, bias=b2_sb[:]
                    )
                    nc.sync.dma_start(out[:, sl], ysb[:])

            if repeat == 1:
                body_bc()
                tail()
            elif repeat > 1:
                # collectives can't live inside For_i (mesh desync), so the
                # timing loop covers phases B+C only; the tail runs once.
                with tc.For_i(0, repeat, 1) as _i:
                    body_bc(_i)
                tail()
            else:
                # repeat == -K: K statically-unrolled tails (for tail timing)
                body_bc()
                for _ in range(-repeat):
                    tail()

    nc.compile()
    return nc


def make_in_maps(X, e, Ri, Ro, W1, b1, W2, b2):
    """Shard + lay out the full inputs for the 8 cores."""
    X = np.asarray(X, dtype=np.float32)
    e = np.asarray(e, dtype=np.float32)
    W1 = np.asarray(W1, dtype=np.float32)
    b1 = np.asarray(b1, dtype=np.float32)
    W2 = np.asarray(W2, dtype=np.float32)
    b2 = np.asarray(b2, dtype=np.float32)

    w1ab = np.ascontiguousarray(W1[:128])                       # [128, OUT]
    w1cp = np.concatenate([W1[128:], np.zeros((64, OUT), np.float32)], axis=0)
    w2p = np.concatenate([W2, np.zeros((64, OUT), np.float32)], axis=0)
    b1c = np.ascontiguousarray(b1.reshape(OUT, 1))
    b2c = np.ascontiguousarray(b2.reshape(OUT, 1))

    in_maps = []
    per_batch = {}
    for b_ in range(B):
        xb = np.asarray(X[b_])
        x16 = xb.astype(np.float16)
        # [p, nb, d] layout so the DMA is fully contiguous
        x16r = np.ascontiguousarray(
            x16.reshape(NB, 128, D).transpose(1, 0, 2)
        ).reshape(128, NB * D)
        xt = np.ascontiguousarray(xb.T)                         # [D, N] f32
        xt2v = np.concatenate([xt, xt], axis=0) * 0.25          # [128, N]
        per_batch[b_] = (x16r, xt2v)

    for c in range(NCORES):
        b_, s = divmod(c, G)
        sl = slice(s * ESH, (s + 1) * ESH)
        ris_ = np.asarray(Ri[b_, :, sl], dtype=np.float32).astype(np.float16)
        ros_ = np.asarray(Ro[b_, :, sl], dtype=np.float32).astype(np.float16)
        ritm = np.ascontiguousarray(ris_.T)
        rotm = np.ascontiguousarray(ros_.T)
        e_sl = np.asarray(e[b_, sl], dtype=np.float32)
        earr = np.ascontiguousarray(e_sl.reshape(ESH // 128, 128).T)
        x16r, xt2v = per_batch[b_]
        in_maps.append(
            {
                "ris": ris_, "ros": ros_, "rit": ritm, "rot": rotm,
                "x16r": x16r, "xt2": xt2v, "earr": earr,
                "w1ab": w1ab, "w1cp": w1cp, "w2p": w2p,
                "b1d": b1c, "b2d": b2c,
            }
        )
    return in_maps


def assemble_output(results):
    y = np.empty((B, N, OUT), dtype=np.float32)
    for c in range(NCORES):
        b_, s = divmod(c, G)
        y[b_, s * NSL : (s + 1) * NSL, :] = results[c]["out"].T
    return y


def get_program(repeat=1):
    key = ("nc", repeat)
    if key not in _cache:
        _cache[key] = _build_program(repeat)
    return _cache[key]


def kernel(X, e, Ri, Ro, W1, b1, W2, b2):
    nc = get_program()
    in_maps = make_in_maps(X, e, Ri, Ro, W1, b1, W2, b2)
    res = run_bass_kernel_spmd(nc, in_maps, list(range(NCORES)))
    return assemble_output(res.results)
